# revision 57
# baseline (speedup 1.0000x reference)
"""Distributed causal multi-head attention kernel for 8 TRN2 NeuronCores.

Problem: B=2, S=2048, D=1024, H=16 heads (hd=64), f32 I/O, causal softmax.
Sharding: data-parallel over batch (2 groups of 4 cores), tensor-parallel over
heads within each group (4 heads/core) and over the wo contraction (row
parallel): each core computes a full-dout partial of F^T from its local heads
and a ReduceScatter(add) hands every core its dout shard of the output.

The collective cost is overhead-dominated (~15us fixed per op + out/40GBps),
so the schedule pipelines per q-chunk: proj(c) -> attn(c) -> wo(c)+RS(c).
RS(0) triggers ~35us in (vs ~108us when all projections run first), so only
the final chunk's RS is exposed at the tail.

Per-core compute (all transposed layouts, bf16 matmul, f32 PSUM accum):
  Q^T,K^T = wqT/wkT_shard.T @ x^T      [256, 2048]
  V       = x @ wv_shard.T             [2048, 4x(64+1)]  (ones col for denom)
  S^T     = K_h @ Q_h^T (per head, software-pipelined with exp on ACT)
  P~      = exp(S^T/8) * causal_mask   (ACT exp, DVE mask mul on diag block)
  Y_h^T,den = [V_h|1].T @ P~           (AV matmul, row 64 = softmax denom)
  yT      = Y_h^T / den                (written into 2 [128,S] head-pair tiles)
  Fpart^T = woT_rows.T @ yT_local      [1024, qc] partial per q-chunk
  ReduceScatter(add) over the 4-core group -> out[256, qc] bf16 per chunk

PSUM budget (8 banks of 2KB/partition):
  pj ring (qp/kp/vp/wp, [128,512] f32, bufs=2)  2 banks
  sc ring ([128,1024] f32, bufs=2)              4 banks
  avA+avB ([128,512] f32, bufs=1 each)          2 banks
avA/avB free early: norm first copies PSUM->SBUF, then does the reciprocal
chain from SBUF, so the next group's AV accumulation isn't gated on the norm.
"""
import numpy as np
import ml_dtypes

import concourse.bass as bass
import concourse.bacc as bacc
import concourse.tile as tile
from concourse import mybir
from concourse.bass import ts, _add_dep_helper

B, S, D, H = 2, 2048, 1024, 16
HD = D // H            # 64
N_CORES = 8
TP = 4                 # cores per batch group
HPC = H // TP          # heads per core = 4
DPC = D // TP          # 256: head-dims per core, also wo dout shard
GROUPS = [[0, 1, 2, 3], [4, 5, 6, 7]]
QC = 512               # q-chunk (free dim of scores)
KT = 128               # k-tile (partition dim of scores)
NQC = S // QC          # 4
NKT = S // KT          # 16
NK = D // 128          # 8 contraction tiles for projections
NM = D // 128          # 8 dout tiles of the wo partial

BF16 = mybir.dt.bfloat16
F32 = mybir.dt.float32


def build():
    nc = bacc.Bacc(None, target_bir_lowering=False, debug=False)

    xT = nc.declare_dram_parameter("xT", [D, S], BF16, isOutput=False)
    wqT = nc.declare_dram_parameter("wqT", [D, DPC], BF16, isOutput=False)
    wkT = nc.declare_dram_parameter("wkT", [D, DPC], BF16, isOutput=False)
    wvT = nc.declare_dram_parameter("wvT", [D, DPC], BF16, isOutput=False)
    # row shard of wo.T: [local din 256, full dout 1024]
    woT = nc.declare_dram_parameter("woT", [DPC, D], BF16, isOutput=False)
    masks = nc.declare_dram_parameter("masks", [4, KT, QC], BF16, isOutput=False)
    # chunk-major so each q-chunk's ReduceScatter output is contiguous
    out = nc.declare_dram_parameter("out", [NQC, DPC, QC], BF16, isOutput=True)

    with tile.TileContext(nc) as tc:
        with (
            tc.tile_pool(name="persist", bufs=1) as persist,
            tc.tile_pool(name="xtp", bufs=1) as xtp,
            tc.tile_pool(name="ptile", bufs=4) as ptile,
            tc.tile_pool(name="pw", bufs=2) as pwp,
            tc.tile_pool(name="norm", bufs=2) as norm,
            tc.tile_pool(name="dram", bufs=1, space="DRAM") as dram,
            tc.tile_pool(name="pj_ps", bufs=2, space="PSUM") as pj_ps,
            tc.tile_pool(name="sc_ps", bufs=2, space="PSUM") as sc_ps,
            tc.tile_pool(name="av_ps", bufs=1, space="PSUM") as av_ps,
        ):
            # ---- persistent SBUF ----
            xt = [xtp.tile([128, S], BF16, name=f"xt{k}") for k in range(NK)]
            wq_s3 = persist.tile([128, NK, DPC], BF16, name="wq_s3")
            wk_s3 = persist.tile([128, NK, DPC], BF16, name="wk_s3")
            wv_s3 = persist.tile([128, NK, DPC], BF16, name="wv_s3")
            wo_s3 = persist.tile([128, 2, D], BF16, name="wo_s3")
            wq_s = [wq_s3[:, k, :] for k in range(NK)]
            wk_s = [wk_s3[:, k, :] for k in range(NK)]
            wv_s = [wv_s3[:, k, :] for k in range(NK)]
            mask_all = persist.tile([KT, 4, QC], BF16, name="mask_all")
            mask_t = [mask_all[:, r, :] for r in range(4)]
            qT = [persist.tile([128, S], BF16, name=f"qT{hp}") for hp in range(2)]
            kT = [persist.tile([128, S], BF16, name=f"kT{hp}") for hp in range(2)]
            vt = [persist.tile([128, HPC, HD + 1], BF16, name=f"v{st}") for st in range(NKT)]
            # yT packed as head pairs: yTp[p] rows 0:64 = head 2p, 64:128 = head 2p+1
            yTp = [persist.tile([128, S], BF16, name=f"yTp{p}") for p in range(2)]

            # tiny dummy AllGather FIRST: the CC cores take ~65us to come up,
            # so trigger the first collective immediately (memset on gpsimd,
            # not vector: the Tile scheduler places vector ops by issue slot
            # and can push a vector memset behind the proj copies, which
            # delays the AG trigger past the CC init and cascades the whole
            # serialized RS chain). Also absorbs inter-core startup skew.
            preamble_sync = [None]
            sync_in = dram.tile([1, 16], BF16, name="sync_in")
            sync_sb = persist.tile([1, 16], BF16, name="sync_sb")
            sync_out = dram.tile([4, 16], BF16, name="sync_out")
            # dummy AllGather with its ENTIRE dependency chain on the gpsimd
            # queue (memset -> staging DMA -> AG): same-queue FIFO order, so
            # no cross-queue scheduling can delay the trigger past the ~60us
            # CC-core init window (which cascades the serialized RS chain;
            # observed repeatedly with a sync-queue staged sync_in)
            nc.gpsimd.memset(sync_sb, 0.0)
            nc.gpsimd.dma_start(out=sync_in, in_=sync_sb)
            ag = nc.gpsimd.collective_compute(
                "AllGather",
                mybir.AluOpType.bypass,
                replica_groups=GROUPS,
                ins=[sync_in[:].opt()],
                outs=[sync_out[:].opt()],
            )

            # first compute needs wq[k] + xt[k] chunk 0 pairwise as the
            # k-accumulation advances: interleave per-k so the first Q
            # matmul starts ~2us in and is DMA-paced, not DMA-blocked
            for k in range(NK):
                wq_dma = nc.gpsimd.dma_start(
                    out=wq_s3[:, k, :], in_=wqT[ts(k, 128), :]
                )
                if k == 0:
                    _add_dep_helper(
                        wq_dma.ins, ag.ins, sync=False,
                        reason="AG trigger issues before the preamble DMAs",
                    )
                x0_dma = nc.sync.dma_start(
                    out=xt[k][:, ts(0, QC)], in_=xT[ts(k, 128), ts(0, QC)]
                )
                preamble_sync[0] = x0_dma
            nc.gpsimd.dma_start(
                out=wk_s3, in_=wkT.rearrange("(k p) d -> p k d", p=128)
            )
            nc.gpsimd.dma_start(
                out=wv_s3, in_=wvT.rearrange("(k p) d -> p k d", p=128)
            )
            nc.gpsimd.dma_start(out=mask_all, in_=masks.rearrange("r p q -> p r q"))
            # x chunks 1-3: on the SYNC queue, strictly AFTER the x0 tiles --
            # the first matmul needs only wq+x0 (1.5MB), and letting these
            # 3MB run concurrently on another queue's DMA engines starves
            # x0's bandwidth and delays the first matmul by ~10us
            for k in range(NK):
                nc.sync.dma_start(
                    out=xt[k][:, QC:S], in_=xT[ts(k, 128), QC:S]
                )
            nc.gpsimd.dma_start(
                out=wo_s3, in_=woT.rearrange("(k p) d -> p k d", p=128)
            )

            # warm up the ACT exp table set during the DMA preamble so the
            # first real exp doesn't pay the ~2.7us ACT_TABLE_LOAD
            warm = persist.tile([128, 16], F32, name="warm")
            nc.vector.memset(warm, 0.0)
            warm2 = persist.tile([128, 16], F32, name="warm2")
            nc.scalar.activation(
                warm2, warm, mybir.ActivationFunctionType.Exp, scale=1.0
            )

            # head-half selector for the PE-side reciprocal broadcast:
            # sel2.T @ [recipA; recipB] puts recipA on partitions 0:64 and
            # recipB on 64:128 (the gpsimd partition_broadcast used before
            # sits on the same queue as the collectives, which BLOCKS until
            # each RS completes -- any norm work there stalls the woven wo
            # matmuls and head-blocks the whole PE queue)
            ones64 = persist.tile([1, HD], BF16, name="ones64")
            nc.vector.memset(ones64, 1.0)

            rs_insts = []
            rso_copies = []

            # ---- filler: proj/wo matmuls woven into the attention stream
            # one closure = one PE instruction (or one small finish op), so
            # the consumption rate between s/av ops can be tuned to keep the
            # ACT exp stream saturated while the PE also retires proj/wo work
            filler = []

            def feed(n):
                for _ in range(n):
                    if filler:
                        filler.pop(0)()

            def accum_closures(mk_tile, mk_mm, n, fin):
                st_ = {}
                cl = []

                def first(st_=st_):
                    st_["t"] = mk_tile()
                    mk_mm(st_["t"], 0)

                cl.append(first)
                for k in range(1, n):
                    cl.append(lambda k=k, st_=st_: mk_mm(st_["t"], k))
                cl.append(lambda st_=st_: fin(st_["t"]))
                return cl

            def proj_closures(nq):
                cl = []
                for m in range(2):  # dout 128-tiles of the 256 local dims
                    cl += accum_closures(
                        lambda nq=nq, m=m: pj_ps.tile(
                            [128, QC], F32, tag="pj", name=f"qp{nq}_{m}"
                        ),
                        lambda t, k, nq=nq, m=m: nc.tensor.matmul(
                            t, wq_s[k][:, ts(m, 128)], xt[k][:, ts(nq, QC)],
                            start=(k == 0), stop=(k == NK - 1),
                        ),
                        NK,
                        lambda t, nq=nq, m=m: nc.vector.tensor_copy(
                            qT[m][:, ts(nq, QC)], t
                        ),
                    )
                    cl += accum_closures(
                        lambda nq=nq, m=m: pj_ps.tile(
                            [128, QC], F32, tag="pj", name=f"kp{nq}_{m}"
                        ),
                        lambda t, k, nq=nq, m=m: nc.tensor.matmul(
                            t, wk_s[k][:, ts(m, 128)], xt[k][:, ts(nq, QC)],
                            start=(k == 0), stop=(k == NK - 1),
                        ),
                        NK,
                        lambda t, nq=nq, m=m: nc.vector.tensor_copy(
                            kT[m][:, ts(nq, QC)], t
                        ),
                    )
                for sm in range(4):  # s-tiles of 128 inside this q-chunk
                    st = nq * 4 + sm

                    def fin_v(t, st=st):
                        nc.vector.memset(vt[st][:, :, HD:HD + 1], 1.0)
                        nc.vector.tensor_copy(
                            vt[st][:, :, 0:HD],
                            t[:, 0:DPC].rearrange("p (h d) -> p h d", h=HPC),
                        )

                    cl += accum_closures(
                        lambda st=st: pj_ps.tile(
                            [128, QC], F32, tag="pj", name=f"vp{st}"
                        ),
                        lambda t, k, st=st: nc.tensor.matmul(
                            t[:, 0:DPC], xt[k][:, ts(st, 128)], wv_s[k],
                            start=(k == 0), stop=(k == NK - 1),
                        ),
                        NK,
                        fin_v,
                    )
                return cl

            def wo_closures(qc):
                """Full-dout partial of F^T for q-chunk qc, staged to DRAM
                in two halves, then one ReduceScatter(add) per chunk
                (collectives serialize on one CC stream at ~15us fixed
                overhead each, so exactly one RS per chunk)."""
                st_ = {}
                cl = []

                def open_(st_=st_, qc=qc):
                    # the second head-pair's normalize finish rides as the
                    # first wo closure: late enough that its reciprocal
                    # (queued on DVE at norm_copy) is done, but strictly
                    # before the wp matmuls that read yTp
                    norm_fin((qc, 1))
                    st_["pt"] = dram.tile([D, QC], BF16, name=f"part{qc}")
                    st_["ro"] = dram.tile([DPC, QC], BF16, name=f"rso{qc}")
                    st_["pw"] = pwp.tile(
                        [128, NM, QC], BF16, tag="pw", name=f"pw{qc}"
                    )

                cl.append(open_)
                for m in range(NM):

                    def fin_w(t, m=m, st_=st_):
                        nc.vector.tensor_copy(st_["pw"][:, m, :], t)
                        if m in (NM // 2 - 1, NM - 1):
                            h = m // (NM // 2)
                            pt_r = st_["pt"].rearrange("(m p) q -> p m q", p=128)
                            pt_dma = nc.sync.dma_start(
                                out=pt_r[:, ts(h, NM // 2), :],
                                in_=st_["pw"][:, ts(h, NM // 2), :],
                            )
                            # pt staging waits on pw copies; if the scheduler
                            # hoists it above the preamble sync-queue DMAs it
                            # head-blocks sync_in and the AG trigger slips to
                            # ~70us (cascading the serialized RS chain)
                            _add_dep_helper(
                                pt_dma.ins, preamble_sync[0].ins, sync=False,
                                reason="pt staging issues after preamble DMAs",
                            )

                    cl += accum_closures(
                        lambda qc=qc, m=m: pj_ps.tile(
                            [128, QC], F32, tag="pj", name=f"wp{qc}_{m}"
                        ),
                        lambda t, k, qc=qc, m=m: nc.tensor.matmul(
                            t, wo_s3[:, k, ts(m, 128)], yTp[k][:, ts(qc, QC)],
                            start=(k == 0), stop=(k == 1),
                        ),
                        2,
                        fin_w,
                    )

                def rs_(st_=st_, qc=qc):
                    cc = nc.gpsimd.collective_compute(
                        "ReduceScatter",
                        mybir.AluOpType.add,
                        replica_groups=GROUPS,
                        ins=[st_["pt"][:].opt()],
                        outs=[st_["ro"][:].opt()],
                    )
                    rs_insts.append(cc)
                    rso_copies.append((qc, st_["ro"]))

                cl.append(rs_)
                return cl

            # ---- attention: one global software-pipelined stream ----
            gstate = {}
            last_act = [None]  # last exp instruction, to pin the out copies

            def make_group(g):
                qc, hp = g
                gstate[g] = {
                    "avA": av_ps.tile([128, QC], F32, tag="avA", name=f"avA{qc}_{hp}"),
                    "avB": av_ps.tile([128, QC], F32, tag="avB", name=f"avB{qc}_{hp}"),
                    "pts": {}, "offs": {}, "n_k": (qc + 1) * 4,
                }

            def s_op(g, m):
                qc, hp = g
                st = gstate[g]
                # causal: columns j < off are fully masked for this k-tile
                off = max(0, (m - 4 * qc) * 128)
                st["offs"][m] = off
                sc = sc_ps.tile([128, 2 * QC], F32, tag="sc", name=f"sc{qc}_{hp}_{m}")
                nc.tensor.matmul(
                    sc[:, off:QC],
                    kT[hp][0:64, ts(m, 128)],
                    qT[hp][0:64, qc * QC + off:(qc + 1) * QC],
                    start=True, stop=True,
                )
                nc.tensor.matmul(
                    sc[:, QC + off:2 * QC],
                    kT[hp][64:128, ts(m, 128)],
                    qT[hp][64:128, qc * QC + off:(qc + 1) * QC],
                    start=True, stop=True,
                )
                pt = ptile.tile([128, 2 * QC], BF16, tag="pt", name=f"pt{qc}_{hp}_{m}")
                # the ACT engine is the attention bottleneck (~120us of exp):
                # full tiles fuse both head-halves into ONE exp op to save
                # the ~400ns fixed overhead; diagonal tiles (off>0, halves
                # non-contiguous) keep two ops
                if off == 0:
                    last_act[0] = nc.scalar.activation(
                        pt, sc,
                        mybir.ActivationFunctionType.Exp,
                        scale=1.0 / np.sqrt(HD),
                    )
                else:
                    for t in range(2):
                        last_act[0] = nc.scalar.activation(
                            pt[:, t * QC + off:(t + 1) * QC],
                            sc[:, t * QC + off:(t + 1) * QC],
                            mybir.ActivationFunctionType.Exp,
                            scale=1.0 / np.sqrt(HD),
                        )
                if m >= 4 * qc:
                    # only the 128-col diagonal block has a mixed mask;
                    # columns >= off+128 are fully unmasked
                    ri = m - 4 * qc
                    for t in range(2):
                        nc.vector.tensor_mul(
                            pt[:, t * QC + off:t * QC + off + 128],
                            pt[:, t * QC + off:t * QC + off + 128],
                            mask_t[ri][:, off:off + 128],
                        )
                st["pts"][m] = pt

            def av_op(g, m):
                qc, hp = g
                st = gstate[g]
                off = st["offs"][m]
                pt = st["pts"].pop(m)
                n_k = st["n_k"]
                nc.tensor.matmul(
                    st["avA"][0:HD + 1, off:QC], vt[m][:, 2 * hp, :],
                    pt[:, off:QC],
                    start=(m == 0), stop=(m == n_k - 1),
                )
                nc.tensor.matmul(
                    st["avB"][0:HD + 1, off:QC], vt[m][:, 2 * hp + 1, :],
                    pt[:, QC + off:2 * QC],
                    start=(m == 0), stop=(m == n_k - 1),
                )

            def norm_copy(g):
                """DVE-only first phase, emitted at the next group's m==1:
                copy the AV accumulators out of PSUM (freeing the banks) and
                run the reciprocal chain so norm_fin's PE op never stalls."""
                qc, hp = g
                st = gstate[g]
                avsA = norm.tile([HD + 1, QC], F32, tag="avsA", name=f"avsA{qc}_{hp}")
                avsB = norm.tile([HD + 1, QC], F32, tag="avsB", name=f"avsB{qc}_{hp}")
                nc.vector.tensor_copy(avsA, st["avA"][0:HD + 1, :])
                nc.vector.tensor_copy(avsB, st["avB"][0:HD + 1, :])
                # reciprocal is a table op: inputs must sit at partition 0
                # (partition-shifting reads work for plain copies but not for
                # the approx-reciprocal path on hardware); both heads' denoms
                # ride in one [1, 2*QC] row
                dnr = norm.tile([1, 2 * QC], F32, tag="dnr", name=f"dnr{qc}_{hp}")
                nc.vector.tensor_copy(dnr[:, 0:QC], avsA[HD:HD + 1, :])
                nc.vector.tensor_copy(dnr[:, QC:2 * QC], avsB[HD:HD + 1, :])
                rcf = norm.tile([1, 2 * QC], F32, tag="rcf", name=f"rcf{qc}_{hp}")
                nc.vector.reciprocal_approx_fast(rcf, dnr)
                rc = norm.tile([1, 2 * QC], BF16, tag="rc", name=f"rc{qc}_{hp}")
                nc.vector.tensor_copy(rc, rcf)
                st["avsA"], st["avsB"], st["rc"] = avsA, avsB, rc

            def norm_fin(g):
                """PE broadcast of the reciprocals + the yT normalize muls;
                emitted one group later so the reciprocals are long done."""
                qc, hp = g
                st = gstate[g]
                rb = pj_ps.tile([128, QC], F32, tag="pj", name=f"rb{qc}_{hp}")
                nc.tensor.matmul(
                    rb[0:HD, :], ones64, st["rc"][:, 0:QC], start=True, stop=True
                )
                nc.tensor.matmul(
                    rb[HD:128, :], ones64, st["rc"][:, QC:2 * QC],
                    start=True, stop=True,
                )
                for hh, avs in ((2 * hp, st["avsA"]), (2 * hp + 1, st["avsB"])):
                    nc.vector.tensor_mul(
                        yTp[hh // 2][(hh % 2) * HD:(hh % 2 + 1) * HD, ts(qc, QC)],
                        avs[0:HD, :], rb[(hh % 2) * HD:(hh % 2 + 1) * HD, :],
                    )

            def attn_stream():
                # ascending chunk order: every RS has the same ~20us cost
                # (inputs are all [D, QC]), so what matters is starting the
                # serialized CC stream early -- chunk 0 finishes ~12us after
                # the projections and its RS absorbs the ~65us CC-core init.
                # (big-first was tried: it delays the first trigger by ~50us
                # and the stream bunches at the end.) proj/wo emitted en
                # bloc: weaving them thin jitters each core's RS-trigger
                # chain and every RS absorbs the skew as extra duration
                order = [0, 1, 2, 3]
                groups = [(qc, hp) for qc in order for hp in range(2)]
                pending = []
                for c in proj_closures(0):
                    c()
                for gi, g in enumerate(groups):
                    qc, hp = g
                    make_group(g)
                    for m in range(gstate[g]["n_k"]):
                        s_op(g, m)
                        pending.append((g, m))
                        if m == 1:
                            while pending[0][0] != g:
                                av_op(*pending.pop(0))
                            if gi > 0:
                                norm_copy(groups[gi - 1])
                            if hp == 0 and gi > 0:
                                prev = groups[gi - 1][0]
                                norm_fin((prev, 0))
                                filler.extend(wo_closures(prev))
                                while filler:
                                    feed(1)
                        elif len(pending) > 2:
                            av_op(*pending.pop(0))
                    if hp == 1 and qc + 1 < NQC:
                        filler.extend(proj_closures(qc + 1))
                        while filler:
                            feed(1)
                while pending:
                    av_op(*pending.pop(0))
                norm_copy(groups[-1])
                norm_fin((order[-1], 0))
                filler.extend(wo_closures(order[-1]))
                while filler:
                    feed(1)

            attn_stream()

            # rso -> out copies, explicitly pinned after the LAST
            # ReduceScatter: the Tile scheduler places same-queue
            # instructions by issue slot, not program position, so without
            # this dep an early copy's RS wait head-blocks whatever queue it
            # lands on (observed on both sync and scalar)
            # copies for chunks 0..2 pinned after RS2 (they overlap RS3);
            # only chunk 3's copy trails the last RS. ALSO pinned behind the
            # last exp: the Tile scheduler places same-queue instructions by
            # issue slot, and a copy hoisted mid-queue waiting on its RS
            # head-blocks the exp stream (observed: 15us stall of chunk-3
            # attention)
            for qc, ro in rso_copies:
                cp = nc.scalar.dma_start(out=out[qc], in_=ro)
                anchor = rs_insts[-2] if qc < NQC - 1 else rs_insts[-1]
                _add_dep_helper(
                    cp.ins, anchor.ins, sync=False,
                    reason="out copies scheduled after late RS",
                )
                _add_dep_helper(
                    cp.ins, last_act[0].ins, sync=False,
                    reason="out copies issue after the exp stream",
                )

    nc.finalize()
    return nc


def make_masks():
    i = np.arange(KT)[:, None]
    j = np.arange(QC)[None, :]
    m = np.zeros((4, KT, QC), dtype=ml_dtypes.bfloat16)
    for r in range(4):
        m[r] = ((r * KT + i) <= j).astype(ml_dtypes.bfloat16)
    return m


def shard_inputs(x, wq, wk, wv, wo):
    """Full f32 inputs -> per-core in_maps (bf16)."""
    bf = ml_dtypes.bfloat16
    masks = make_masks()
    wqT = np.ascontiguousarray(wq.T).astype(bf)
    wkT = np.ascontiguousarray(wk.T).astype(bf)
    wvT = np.ascontiguousarray(wv.T).astype(bf)
    woT = np.ascontiguousarray(wo.T).astype(bf)
    in_maps = []
    for c in range(N_CORES):
        b, tp = divmod(c, TP)
        sl = slice(tp * DPC, (tp + 1) * DPC)
        in_maps.append({
            "xT": np.ascontiguousarray(x[b].T).astype(bf),
            "wqT": np.ascontiguousarray(wqT[:, sl]),
            "wkT": np.ascontiguousarray(wkT[:, sl]),
            "wvT": np.ascontiguousarray(wvT[:, sl]),
            "woT": np.ascontiguousarray(woT[sl, :]),
            "masks": masks,
        })
    return in_maps


def assemble_output(results):
    """Per-core F^T shards [NQC, DPC, QC] bf16 -> full [B, S, D] f32."""
    outs = []
    for b in range(B):
        ft = np.concatenate(
            [
                np.concatenate(list(results[b * TP + tp]["out"]), axis=1)
                for tp in range(TP)
            ],
            axis=0,
        ).astype(np.float32)  # [D, S]
        outs.append(ft.T)  # [S, D]
    return np.stack(outs, axis=0)


_NC_CACHE = []


def kernel(x, wq, wk, wv, wo):
    """Full-input distributed attention on 8 NeuronCores; returns full output."""
    x = np.asarray(x, dtype=np.float32)
    wq = np.asarray(wq, dtype=np.float32)
    wk = np.asarray(wk, dtype=np.float32)
    wv = np.asarray(wv, dtype=np.float32)
    wo = np.asarray(wo, dtype=np.float32)
    if not _NC_CACHE:
        _NC_CACHE.append(build())
    nc = _NC_CACHE[0]
    in_maps = shard_inputs(x, wq, wk, wv, wo)
    from concourse import bass2jax
    results = bass2jax.run_bass_via_pjrt(nc, in_maps, n_cores=N_CORES)
    return assemble_output(results).astype(np.float32)
